# revision 31
# baseline (speedup 1.0000x reference)
"""Causal linear attention (ELU+1 feature map) on 8 trn2 NeuronCores. v2.

Sharding: core i handles batch b=i//2, sequence half h=i%2 (T=2048 -> 1024
tokens/core).  Second-half cores recompute the first half's running state
S0 = sum_tau phi(k_tau) [v_tau, 1]  (128x129, col 128 = z) from k/v of the
first half; first-half cores get zeroed aux inputs so their S0 == 0.

phi(y) = ELU(y)+1 = min(exp(y), max(y+1, 1))  (exact identity).
pre' = y + b + 1 computed in PSUM via matmul (bias row via ones-mm), then
  e = ACT Exp(pre' - 1);  phi = DVE stt: min(max(pre', 1), e).

q/k/W shipped fp8e4m3 (rel-err impact ~+7e-4), v/mask/ident bf16, out bf16.
"""

import numpy as np

B, T, D, DV = 4, 2048, 128, 128
H = T // 2          # tokens per core
C = 128             # chunk
NCH = H // C        # own chunks per core
NCORES = 8
VW = DV + 1

# fp8 pack: [WT | ktp | kT | qT]
O8_WT = 0
O8_KTP = O8_WT + D
O8_KT = O8_KTP + H
O8_QT = O8_KT + H
B8_COLS = O8_QT + H

# bf16 pack: [mask | ident | bias | vp | v]
O16_MASK = 0
O16_ID = O16_MASK + C
O16_BIAS = O16_ID + C
O16_VP = O16_BIAS + 1
O16_V = O16_VP + NCH * VW
B16_COLS = O16_V + NCH * VW

CFG = {
    "stt_eng": ("dve", "dve", "dve", "dve", "dve", "dve"),  # dve | poolsbuf
    "snap_eng": ("act",) * 8,
    "ktok_eng": ("dve", "dve"),   # per half copy
    "mask_eng": ("dve", "dve"),
    "scale_mode": "recip_dve",    # div_dve | recip_dve
    "dma_map": ("sync", "sync", "sync", "act", "sync", "gpsimd"),
    "out_dma": ("sync", "sync", "sync", "sync"),
    "phi_order": ("t0", "k0", "q0", "t1", "k1", "q1"),
    "chunk_grp": 2,
    "in_fp8": True,
    "out_bf16": True,
}

_cache = {}


def _build(cfg=None):
    import concourse.bacc as bacc
    import concourse.tile as tile
    from concourse import mybir
    from bass_rust import add_dep_helper

    cfg = dict(CFG, **(cfg or {}))
    F32 = mybir.dt.float32
    BF16 = mybir.dt.bfloat16
    FP8 = mybir.dt.float8e4
    AF = mybir.ActivationFunctionType
    ALU = mybir.AluOpType

    nc = bacc.Bacc(None, target_bir_lowering=False, debug=False,
                   num_devices=NCORES)

    DT8 = FP8 if cfg["in_fp8"] else BF16
    DTO = BF16 if cfg["out_bf16"] else F32
    b8 = nc.declare_dram_parameter("b8", [D, B8_COLS], DT8, isOutput=False)
    b16 = nc.declare_dram_parameter("b16", [D, B16_COLS], BF16,
                                    isOutput=False)
    bsm = nc.declare_dram_parameter("bsm", [2, 512], BF16, isOutput=False)
    out = nc.declare_dram_parameter("out", [C, NCH * DV], DTO, isOutput=True)
    dbg = nc.declare_dram_parameter("dbg", [C, 3 * H], BF16, isOutput=True) \
        if cfg.get("debug_phi") else None

    def eng(which):
        return {"dve": nc.vector, "act": nc.scalar, "pool": nc.gpsimd,
                "gpsimd": nc.gpsimd, "sync": nc.sync}[which]

    def copy_eng(which, dst, src):
        if which == "act":
            nc.scalar.activation(dst, src, AF.Copy)
        else:
            eng(which).tensor_copy(dst, src)

    with tile.TileContext(nc) as tc:
        with (
            tc.tile_pool(name="cst", bufs=1) as cst,
            tc.tile_pool(name="io", bufs=1) as io,
            tc.tile_pool(name="phi", bufs=1) as phip,
            tc.tile_pool(name="wrk", bufs=2) as wrk,
            tc.tile_pool(name="ps_pre", bufs=4, space="PSUM") as ps_pre,
            tc.tile_pool(name="ps_s", bufs=1, space="PSUM") as ps_s,
            tc.tile_pool(name="ps_o", bufs=2, space="PSUM") as ps_o,
        ):
            # ---- warm the ACT table while DMAs run ----
            s_warm = cst.tile([D, 1], F32)
            nc.vector.memset(s_warm, 0.0)
            s_warm2 = cst.tile([D, 1], BF16)
            nc.scalar.activation(s_warm2, s_warm, AF.Exp)

            onesw = cst.tile([2, 512], BF16)
            nc.gpsimd.memset(onesw, 1.0)
            s_neg1 = cst.tile([D, 1], F32)
            nc.gpsimd.memset(s_neg1, -1.0)

            # ---- loads (need-ordered; SP carries the phi-critical fp8) ----
            s_b8 = io.tile([D, B8_COLS], DT8)
            s_b16 = io.tile([D, B16_COLS], BF16)
            sb2 = io.tile([2, 512], BF16)
            nc.gpsimd.dma_start(out=sb2[:, :], in_=bsm[:, :])
            HH = 512
            sp_pieces = [
                (0, O8_KTP + HH),                  # W + ktp half 0
                (O8_KT, O8_KT + HH),               # kT half 0
                (O8_QT, O8_QT + HH),               # qT half 0
                (O8_KTP + HH, O8_KT),              # ktp half 1
                (O8_KT + HH, O8_QT),               # kT half 1
                (O8_QT + HH, B8_COLS),             # qT half 1
            ]
            for a, b_ in sp_pieces:
                nc.sync.dma_start(out=s_b8[:, a:b_], in_=b8[:, a:b_])
            nc.scalar.dma_start(out=s_b16[:, 0:O16_VP],
                                in_=b16[:, 0:O16_VP])
            nc.scalar.dma_start(out=s_b16[:, O16_VP:O16_V],
                                in_=b16[:, O16_VP:O16_V])
            nc.gpsimd.dma_start(out=s_b16[:, O16_V:B16_COLS],
                                in_=b16[:, O16_V:B16_COLS])

            sWT = s_b8[:, O8_WT:O8_WT + D]
            s_mask = s_b16[:, O16_MASK:O16_MASK + C]
            s_ident = s_b16[:, O16_ID:O16_ID + C]
            s_bias = s_b16[:, O16_BIAS:O16_BIAS + 1]

            def vsl(c):
                return s_b16[:, O16_V + VW * c:O16_V + VW * (c + 1)]

            def vpsl(c):
                return s_b16[:, O16_VP + VW * c:O16_VP + VW * (c + 1)]

            trp = None  # per-half transpose scratch from the pre ring


            # ---- state S: two parity accumulators ----
            S0 = ps_s.tile([D, VW], F32, tag="s0")
            S1 = ps_s.tile([D, VW], F32, tag="s1")
            Sp = [S0, S1]
            s_first = [None, None]
            started = [False, False]

            phit = phip.tile([C, H], BF16)      # phi(ktp) token-major
            kphi = phip.tile([D, H], BF16)      # phi(k) feature-major
            qphi = phip.tile([D, H], BF16)      # phi(q) feature-major
            ktok = phip.tile([C, H], BF16)      # phi(k) token-major
            Am = phip.tile([C, H], BF16)        # masked A, all chunks
            outst = phip.tile([C, NCH * DV], DTO)

            Ah = [None, None]

            def phi_block(kind, j, sidx, step=0):
                """kind: 'tok'|'k'|'q'; j: half index; sidx: stt engine."""
                pre = ps_pre.tile([D, 512], F32, tag="pre")
                if kind == "tok":
                    prev = nc.tensor.matmul(
                        pre, onesw[:, 0:C], sb2,
                        start=True, stop=False, skip_group_check=True)
                    for cc in range(4):
                        c = 4 * j + cc
                        mm = nc.tensor.matmul(
                            pre[:, C * cc:C * (cc + 1)],
                            s_b8[:, O8_KTP + C * c:O8_KTP + C * (c + 1)],
                            sWT, start=False, stop=(cc == 3),
                            skip_group_check=True)
                        add_dep_helper(mm.ins, prev.ins, sync=False,
                                       reason="psum group order")
                        prev = mm
                else:
                    off = O8_KT if kind == "k" else O8_QT
                    mm0 = nc.tensor.matmul(
                        pre, sWT, s_b8[:, off + 512 * j:off + 512 * (j + 1)],
                        start=True, stop=False, skip_group_check=True)
                    mm = nc.tensor.matmul(
                        pre, sb2[:, 0:C], onesw,
                        start=False, stop=True, skip_group_check=True)
                    add_dep_helper(mm.ins, mm0.ins, sync=False,
                                   reason="psum group order")
                e = phip.tile([D, 512], BF16, tag=f"e{step % 3}")
                nc.scalar.activation(e, pre, AF.Exp, bias=s_neg1, scale=1.0)
                dst = {"tok": phit, "k": kphi, "q": qphi}[kind]
                sl = slice(512 * j, 512 * (j + 1))
                if sidx == "poolsbuf":
                    # gpsimd cannot touch PSUM: relu via ACT, stt on SBUF
                    r = wrk.tile([D, 512], BF16, tag="r")
                    nc.scalar.activation(r, pre, AF.Relu, bias=s_neg1,
                                         scale=1.0)
                    nc.gpsimd.scalar_tensor_tensor(
                        out=dst[:, sl], in0=r, scalar=1.0, in1=e,
                        op0=ALU.add, op1=ALU.min)
                else:
                    eng(sidx).scalar_tensor_tensor(
                        out=dst[:, sl], in0=pre, scalar=1.0, in1=e,
                        op0=ALU.max, op1=ALU.min)

            def pre_s(j):
                for cc in range(4):
                    c = 4 * j + cc
                    p = c % 2
                    mm = nc.tensor.matmul(
                        Sp[p], phit[:, C * c:C * (c + 1)], vpsl(c),
                        start=(not started[p]), stop=False,
                        skip_group_check=True)
                    if started[p]:
                        add_dep_helper(mm.ins, s_first[p].ins, sync=False,
                                       reason="psum group order")
                    s_first[p] = mm
                    started[p] = True

            def transposes(h):
                trp_h = ps_pre.tile([C, 512], BF16, tag="pre")
                for cc in range(4):
                    c = 4 * h + cc
                    nc.tensor.transpose(trp_h[:, C * cc:C * (cc + 1)],
                                        kphi[:, C * c:C * (c + 1)], s_ident)
                sl = slice(512 * h, 512 * (h + 1))
                copy_eng(cfg["ktok_eng"][h], ktok[:, sl], trp_h)

            def a_mms(h):
                A = ps_pre.tile([C, 512], F32, tag="pre")
                Ah[h] = A
                for cc in range(4):
                    c = 4 * h + cc
                    nc.tensor.matmul(A[:, C * cc:C * (cc + 1)],
                                     kphi[:, C * c:C * (c + 1)],
                                     qphi[:, C * c:C * (c + 1)],
                                     start=True, stop=True,
                                     skip_group_check=True)
                sl = slice(512 * h, 512 * (h + 1))
                eng(cfg["mask_eng"][h]).tensor_tensor(
                    out=Am[:, sl].rearrange("p (c f) -> p c f", c=4),
                    in0=A.rearrange("p (c f) -> p c f", c=4),
                    in1=s_mask[:, None, :].broadcast_to([C, 4, C]),
                    op=ALU.mult)

            snaps = [None, None]
            Opair = [None]

            def run_chunk(c):
                pl = (0, 1) if c == 0 else ((c - 1) % 2,)
                for p in pl:
                    snap_t = wrk.tile([D, VW], BF16, tag=f"snap{p}")
                    snp = snap_t
                    snaps[p] = snp
                    copy_eng(cfg["snap_eng"][c], snp, Sp[p])

                if c % 2 == 0:
                    opair_t = ps_o.tile([C, 2 * VW], F32, tag="o")
                    Opair[0] = opair_t
                O = Opair[0]
                base = VW * (c % 2)
                osl = slice(base, base + VW)
                prev = nc.tensor.matmul(O[:, osl], Am[:, C * c:C * (c + 1)],
                                        vsl(c), start=True, stop=False,
                                        skip_group_check=True)
                for qi, sn in enumerate(snaps):
                    mm = nc.tensor.matmul(O[:, osl],
                                          qphi[:, C * c:C * (c + 1)], sn,
                                          start=False, stop=(qi == 1),
                                          skip_group_check=True)
                    add_dep_helper(mm.ins, prev.ins, sync=False,
                                   reason="psum group order")
                    prev = mm

                mm = nc.tensor.matmul(Sp[c % 2],
                                      ktok[:, C * c:C * (c + 1)], vsl(c),
                                      start=False,
                                      stop=(c >= NCH - 2),
                                      skip_group_check=True)
                add_dep_helper(mm.ins, s_first[c % 2].ins, sync=False,
                               reason="psum group order")
                s_first[c % 2] = mm

                if c % 2 == 1:
                    sm = cfg["scale_mode"]
                    for i, cc in enumerate((c - 1, c)):
                        bs = VW * i
                        if sm == "div_dve":
                            nc.vector.tensor_scalar(
                                out=outst[:, DV * cc:DV * (cc + 1)],
                                in0=O[:, bs:bs + DV],
                                scalar1=O[:, bs + DV:bs + DV + 1],
                                scalar2=None, op0=ALU.divide)
                        else:
                            rec = wrk.tile([C, 1], F32, tag="rec")
                            nc.vector.reciprocal(rec, O[:, bs + DV:bs + DV + 1])
                            nc.vector.tensor_scalar_mul(
                                outst[:, DV * cc:DV * (cc + 1)],
                                O[:, bs:bs + DV], rec)
                    oq = cfg["out_dma"][(c - 1) // 2]
                    eng(oq).dma_start(
                        out=out[:, DV * (c - 1):DV * (c + 1)],
                        in_=outst[:, DV * (c - 1):DV * (c + 1)])

            # ---- schedule ----
            done = set()
            emitted = 0

            def pump():
                nonlocal emitted
                if emitted == 0 and {"t0", "t1", "k0", "q0"} <= done:
                    for c in range(cfg["chunk_grp"]):
                        run_chunk(c)
                    emitted = cfg["chunk_grp"]
                elif 0 < emitted < 4 and {"t0", "t1", "k0", "q0"} <= done:
                    for c in range(emitted, 4):
                        run_chunk(c)
                    emitted = 4
                if emitted == 4 and {"k1", "q1"} <= done:
                    for c in range(4, NCH):
                        run_chunk(c)
                    emitted = 8

            for step, blk in enumerate(cfg["phi_order"]):
                kind = {"t": "tok", "k": "k", "q": "q"}[blk[0]]
                j = int(blk[1])
                phi_block(kind, j, cfg["stt_eng"][step], step)
                if kind == "tok":
                    pre_s(j)
                elif kind == "k":
                    transposes(j)
                else:
                    a_mms(j)
                done.add(blk)
                pump()
            assert emitted == 8, "phi_order must cover t0,t1,k0,k1,q0,q1"
            if dbg is not None:
                nc.sync.dma_start(out=dbg[:, 0:H], in_=phit)
                nc.sync.dma_start(out=dbg[:, H:2 * H], in_=ktok)
                nc.sync.dma_start(
                    out=dbg[:, 2 * H:3 * H],
                    in_=kphi.rearrange("p f -> p f"))

    nc.compile()
    return nc


def _get_nc():
    if "nc" not in _cache:
        _cache["nc"] = _build(_cache.get("cfg"))
    return _cache["nc"]


def _pack_inputs(q, k, v, W_phi, b_phi):
    import ml_dtypes
    bf16 = ml_dtypes.bfloat16
    fp8 = ml_dtypes.float8_e4m3fn

    WT = np.ascontiguousarray(W_phi.T)                    # [d, e]
    maskm = np.triu(np.ones((C, C), np.float32))          # keep tau <= t
    ident = np.eye(C, dtype=np.float32)

    def aug(vh):  # [H, DV] -> [C, NCH*(DV+1)] partition-major with ones col
        a = np.concatenate([vh, np.ones((H, 1), np.float32)], axis=1)
        return a.reshape(NCH, C, VW).transpose(1, 0, 2).reshape(C, NCH * VW)

    zeros_vp = np.zeros((C, NCH * VW), np.float32)
    zeros_ktp = np.zeros((D, H), np.float32)

    in_maps = []
    for core in range(NCORES):
        b_idx, half = divmod(core, 2)
        sl = slice(half * H, (half + 1) * H)
        a8 = np.empty((D, B8_COLS), np.float32)
        a8[:, O8_WT:O8_WT + D] = WT
        a8[:, O8_QT:O8_QT + H] = q[b_idx, sl].T
        a8[:, O8_KT:O8_KT + H] = k[b_idx, sl].T
        a16 = np.empty((D, B16_COLS), np.float32)
        a16[:, O16_MASK:O16_MASK + C] = maskm
        a16[:, O16_ID:O16_ID + C] = ident
        a16[:, O16_BIAS] = b_phi
        if half == 1:
            a8[:, O8_KTP:O8_KTP + H] = k[b_idx, 0:H].T
            a16[:, O16_VP:O16_VP + NCH * VW] = aug(v[b_idx, 0:H])
        else:
            a8[:, O8_KTP:O8_KTP + H] = zeros_ktp
            a16[:, O16_VP:O16_VP + NCH * VW] = zeros_vp
        a16[:, O16_V:O16_V + NCH * VW] = aug(v[b_idx, sl])
        dt8 = fp8 if dict(CFG, **(_cache.get("cfg") or {}))["in_fp8"] else bf16
        bsm = np.concatenate([np.tile(b_phi, 4)[None, :],
                              np.ones((1, 512), np.float32)], axis=0)
        in_maps.append({"b8": a8.astype(dt8), "b16": a16.astype(bf16),
                        "bsm": bsm.astype(bf16)})
    return in_maps


def kernel(q, k, v, W_phi, b_phi):
    from concourse.bass_utils import run_bass_kernel_spmd

    q = np.asarray(q, np.float32)
    k = np.asarray(k, np.float32)
    v = np.asarray(v, np.float32)
    W_phi = np.asarray(W_phi, np.float32)
    b_phi = np.asarray(b_phi, np.float32)

    in_maps = _pack_inputs(q, k, v, W_phi, b_phi)
    nc = _get_nc()
    res = run_bass_kernel_spmd(nc, in_maps, list(range(NCORES)))

    out = np.empty((B, T, DV), np.float32)
    for core in range(NCORES):
        b_idx, half = divmod(core, 2)
        o = np.asarray(res.results[core]["out"], dtype=np.float32)
        o = o.reshape(C, NCH, DV).transpose(1, 0, 2).reshape(H, DV)
        out[b_idx, half * H:(half + 1) * H] = o
    return out


# revision 32
# speedup vs baseline: 1.0719x; 1.0719x over previous
"""Causal linear attention (ELU+1 feature map) on 8 trn2 NeuronCores. v2.

Sharding: core i handles batch b=i//2, sequence half h=i%2 (T=2048 -> 1024
tokens/core).  Second-half cores recompute the first half's running state
S0 = sum_tau phi(k_tau) [v_tau, 1]  (128x129, col 128 = z) from k/v of the
first half; first-half cores get zeroed aux inputs so their S0 == 0.

phi(y) = ELU(y)+1 = min(exp(y), max(y+1, 1))  (exact identity).
pre' = y + b + 1 computed in PSUM via matmul (bias row via ones-mm), then
  e = ACT Exp(pre' - 1);  phi = DVE stt: min(max(pre', 1), e).

q/k/W shipped fp8e4m3 (rel-err impact ~+7e-4), v/mask/ident bf16, out bf16.
"""

import numpy as np

B, T, D, DV = 4, 2048, 128, 128
H = T // 2          # tokens per core
C = 128             # chunk
NCH = H // C        # own chunks per core
NCORES = 8
VW = DV + 1

# fp8 pack: [WT | ktp | kT | qT]
O8_WT = 0
O8_KTP = O8_WT + D
O8_KT = O8_KTP + H
O8_QT = O8_KT + H
B8_COLS = O8_QT + H

# bf16 pack: [mask | ident | bias | vp | v]
O16_MASK = 0
O16_ID = O16_MASK + C
O16_BIAS = O16_ID + C
O16_VP = O16_BIAS + 1
O16_V = O16_VP + NCH * VW
B16_COLS = O16_V + NCH * VW

CFG = {
    "stt_eng": ("dve", "dve", "dve", "dve", "dve", "dve"),  # dve | poolsbuf
    "snap_eng": ("act",) * 8,
    "ktok_eng": ("dve", "dve"),   # per half copy
    "mask_eng": ("dve", "dve"),
    "scale_mode": "recip_dve",    # div_dve | recip_dve
    "dma_map": ("sync", "sync", "sync", "act", "sync", "gpsimd"),
    "out_dma": ("sync", "sync", "sync", "sync"),
    "phi_order": ("t0", "k0", "q0", "t1", "k1", "q1"),
    "chunk_grp": 2,
    "in_fp8": True,
    "out_bf16": True,
}

_cache = {}


def _build(cfg=None):
    import concourse.bacc as bacc
    import concourse.tile as tile
    from concourse import mybir
    from bass_rust import add_dep_helper

    cfg = dict(CFG, **(cfg or {}))
    F32 = mybir.dt.float32
    BF16 = mybir.dt.bfloat16
    FP8 = mybir.dt.float8e4
    AF = mybir.ActivationFunctionType
    ALU = mybir.AluOpType

    nc = bacc.Bacc(None, target_bir_lowering=False, debug=False,
                   num_devices=NCORES)

    DT8 = FP8 if cfg["in_fp8"] else BF16
    DTO = BF16 if cfg["out_bf16"] else F32
    b8 = nc.declare_dram_parameter("b8", [D, B8_COLS], DT8, isOutput=False)
    b16 = nc.declare_dram_parameter("b16", [D, B16_COLS], BF16,
                                    isOutput=False)
    bsm = nc.declare_dram_parameter("bsm", [2, 512], BF16, isOutput=False)
    out = nc.declare_dram_parameter("out", [C, NCH * DV], DTO, isOutput=True)
    dbg = nc.declare_dram_parameter("dbg", [C, 3 * H], BF16, isOutput=True) \
        if cfg.get("debug_phi") else None

    def eng(which):
        return {"dve": nc.vector, "act": nc.scalar, "pool": nc.gpsimd,
                "gpsimd": nc.gpsimd, "sync": nc.sync}[which]

    def copy_eng(which, dst, src):
        if which == "act":
            nc.scalar.activation(dst, src, AF.Copy)
        else:
            eng(which).tensor_copy(dst, src)

    with tile.TileContext(nc) as tc:
        with (
            tc.tile_pool(name="cst", bufs=1) as cst,
            tc.tile_pool(name="io", bufs=1) as io,
            tc.tile_pool(name="phi", bufs=1) as phip,
            tc.tile_pool(name="wrk", bufs=2) as wrk,
            tc.tile_pool(name="ps_pre", bufs=3, space="PSUM") as ps_pre,
            tc.tile_pool(name="ps_a", bufs=1, space="PSUM") as ps_a,
            tc.tile_pool(name="ps_t", bufs=1, space="PSUM") as ps_t,
            tc.tile_pool(name="ps_s", bufs=1, space="PSUM") as ps_s,
            tc.tile_pool(name="ps_o", bufs=1, space="PSUM") as ps_o,
        ):
            # ---- warm the ACT table while DMAs run ----
            s_warm = cst.tile([D, 1], F32)
            nc.vector.memset(s_warm, 0.0)
            s_warm2 = cst.tile([D, 1], BF16)
            nc.scalar.activation(s_warm2, s_warm, AF.Exp)

            onesw = cst.tile([2, 512], BF16)
            nc.gpsimd.memset(onesw, 1.0)
            s_neg1 = cst.tile([D, 1], F32)
            nc.gpsimd.memset(s_neg1, -1.0)

            # ---- loads (need-ordered; SP carries the phi-critical fp8) ----
            s_b8 = io.tile([D, B8_COLS], DT8)
            s_b16 = io.tile([D, B16_COLS], BF16)
            sb2 = io.tile([2, 512], BF16)
            nc.gpsimd.dma_start(out=sb2[:, :], in_=bsm[:, :])
            HH = 512
            sp_pieces = [
                (0, O8_KTP + HH),                  # W + ktp half 0
                (O8_KT, O8_KT + HH),               # kT half 0
                (O8_QT, O8_QT + HH),               # qT half 0
                (O8_KTP + HH, O8_KT),              # ktp half 1
                (O8_KT + HH, O8_QT),               # kT half 1
                (O8_QT + HH, B8_COLS),             # qT half 1
            ]
            for a, b_ in sp_pieces:
                nc.sync.dma_start(out=s_b8[:, a:b_], in_=b8[:, a:b_])
            nc.scalar.dma_start(out=s_b16[:, 0:O16_VP],
                                in_=b16[:, 0:O16_VP])
            nc.scalar.dma_start(out=s_b16[:, O16_VP:O16_V],
                                in_=b16[:, O16_VP:O16_V])
            nc.gpsimd.dma_start(out=s_b16[:, O16_V:B16_COLS],
                                in_=b16[:, O16_V:B16_COLS])

            sWT = s_b8[:, O8_WT:O8_WT + D]
            s_mask = s_b16[:, O16_MASK:O16_MASK + C]
            s_ident = s_b16[:, O16_ID:O16_ID + C]
            s_bias = s_b16[:, O16_BIAS:O16_BIAS + 1]

            def vsl(c):
                return s_b16[:, O16_V + VW * c:O16_V + VW * (c + 1)]

            def vpsl(c):
                return s_b16[:, O16_VP + VW * c:O16_VP + VW * (c + 1)]




            # ---- state S: two parity accumulators ----
            S0 = ps_s.tile([D, VW], F32, tag="s0")
            S1 = ps_s.tile([D, VW], F32, tag="s1")
            Sp = [S0, S1]
            s_first = [None, None]
            started = [False, False]

            phit = phip.tile([C, H], BF16)      # phi(ktp) token-major
            kphi = phip.tile([D, H], BF16)      # phi(k) feature-major
            qphi = phip.tile([D, H], BF16)      # phi(q) feature-major
            ktok = phip.tile([C, H], BF16)      # phi(k) token-major
            Am = phip.tile([C, H], BF16)        # masked A, all chunks
            outst = phip.tile([C, NCH * DV], DTO)

            Ah = [None, None]

            def phi_block(kind, j, sidx, step=0):
                """kind: 'tok'|'k'|'q'; j: half index; sidx: stt engine."""
                pre = ps_pre.tile([D, 512], F32, tag="pre")
                if kind == "tok":
                    prev = nc.tensor.matmul(
                        pre, onesw[:, 0:C], sb2,
                        start=True, stop=False, skip_group_check=True)
                    for cc in range(4):
                        c = 4 * j + cc
                        mm = nc.tensor.matmul(
                            pre[:, C * cc:C * (cc + 1)],
                            s_b8[:, O8_KTP + C * c:O8_KTP + C * (c + 1)],
                            sWT, start=False, stop=(cc == 3),
                            skip_group_check=True)
                        add_dep_helper(mm.ins, prev.ins, sync=False,
                                       reason="psum group order")
                        prev = mm
                else:
                    off = O8_KT if kind == "k" else O8_QT
                    mm0 = nc.tensor.matmul(
                        pre, sWT, s_b8[:, off + 512 * j:off + 512 * (j + 1)],
                        start=True, stop=False, skip_group_check=True)
                    mm = nc.tensor.matmul(
                        pre, sb2[:, 0:C], onesw,
                        start=False, stop=True, skip_group_check=True)
                    add_dep_helper(mm.ins, mm0.ins, sync=False,
                                   reason="psum group order")
                e = phip.tile([D, 512], BF16, tag=f"e{step % 3}")
                nc.scalar.activation(e, pre, AF.Exp, bias=s_neg1, scale=1.0)
                dst = {"tok": phit, "k": kphi, "q": qphi}[kind]
                sl = slice(512 * j, 512 * (j + 1))
                if sidx == "poolsbuf":
                    # gpsimd cannot touch PSUM: relu via ACT, stt on SBUF
                    r = wrk.tile([D, 512], BF16, tag="r")
                    nc.scalar.activation(r, pre, AF.Relu, bias=s_neg1,
                                         scale=1.0)
                    nc.gpsimd.scalar_tensor_tensor(
                        out=dst[:, sl], in0=r, scalar=1.0, in1=e,
                        op0=ALU.add, op1=ALU.min)
                else:
                    eng(sidx).scalar_tensor_tensor(
                        out=dst[:, sl], in0=pre, scalar=1.0, in1=e,
                        op0=ALU.max, op1=ALU.min)

            def pre_s(j):
                for cc in range(4):
                    c = 4 * j + cc
                    p = c % 2
                    mm = nc.tensor.matmul(
                        Sp[p], phit[:, C * c:C * (c + 1)], vpsl(c),
                        start=(not started[p]), stop=False,
                        skip_group_check=True)
                    if started[p]:
                        add_dep_helper(mm.ins, s_first[p].ins, sync=False,
                                       reason="psum group order")
                    s_first[p] = mm
                    started[p] = True

            def transposes(h):
                trp_h = ps_t.tile([C, 512], BF16, tag="trp")
                for cc in range(4):
                    c = 4 * h + cc
                    nc.tensor.transpose(trp_h[:, C * cc:C * (cc + 1)],
                                        kphi[:, C * c:C * (c + 1)], s_ident)
                sl = slice(512 * h, 512 * (h + 1))
                copy_eng(cfg["ktok_eng"][h], ktok[:, sl], trp_h)

            def a_mms(h):
                A = ps_a.tile([C, 512], F32, tag="a")
                Ah[h] = A
                for cc in range(4):
                    c = 4 * h + cc
                    nc.tensor.matmul(A[:, C * cc:C * (cc + 1)],
                                     kphi[:, C * c:C * (c + 1)],
                                     qphi[:, C * c:C * (c + 1)],
                                     start=True, stop=True,
                                     skip_group_check=True)
                sl = slice(512 * h, 512 * (h + 1))
                eng(cfg["mask_eng"][h]).tensor_tensor(
                    out=Am[:, sl].rearrange("p (c f) -> p c f", c=4),
                    in0=A.rearrange("p (c f) -> p c f", c=4),
                    in1=s_mask[:, None, :].broadcast_to([C, 4, C]),
                    op=ALU.mult)

            snaps = [None, None]
            Opair = [None]

            def run_chunk(c):
                pl = (0, 1) if c == 0 else ((c - 1) % 2,)
                for p in pl:
                    snap_t = wrk.tile([D, VW], BF16, tag=f"snap{p}")
                    snp = snap_t
                    snaps[p] = snp
                    copy_eng(cfg["snap_eng"][c], snp, Sp[p])

                if c % 2 == 0:
                    opair_t = ps_o.tile([C, 2 * VW], F32, tag="o")
                    Opair[0] = opair_t
                O = Opair[0]
                base = VW * (c % 2)
                osl = slice(base, base + VW)
                prev = nc.tensor.matmul(O[:, osl], Am[:, C * c:C * (c + 1)],
                                        vsl(c), start=True, stop=False,
                                        skip_group_check=True)
                for qi, sn in enumerate(snaps):
                    mm = nc.tensor.matmul(O[:, osl],
                                          qphi[:, C * c:C * (c + 1)], sn,
                                          start=False, stop=(qi == 1),
                                          skip_group_check=True)
                    add_dep_helper(mm.ins, prev.ins, sync=False,
                                   reason="psum group order")
                    prev = mm

                mm = nc.tensor.matmul(Sp[c % 2],
                                      ktok[:, C * c:C * (c + 1)], vsl(c),
                                      start=False,
                                      stop=(c >= NCH - 2),
                                      skip_group_check=True)
                add_dep_helper(mm.ins, s_first[c % 2].ins, sync=False,
                               reason="psum group order")
                s_first[c % 2] = mm

                if c % 2 == 1:
                    sm = cfg["scale_mode"]
                    for i, cc in enumerate((c - 1, c)):
                        bs = VW * i
                        if sm == "div_dve":
                            nc.vector.tensor_scalar(
                                out=outst[:, DV * cc:DV * (cc + 1)],
                                in0=O[:, bs:bs + DV],
                                scalar1=O[:, bs + DV:bs + DV + 1],
                                scalar2=None, op0=ALU.divide)
                        else:
                            rec = wrk.tile([C, 1], F32, tag="rec")
                            nc.vector.reciprocal(rec, O[:, bs + DV:bs + DV + 1])
                            nc.vector.tensor_scalar_mul(
                                outst[:, DV * cc:DV * (cc + 1)],
                                O[:, bs:bs + DV], rec)
                    oq = cfg["out_dma"][(c - 1) // 2]
                    eng(oq).dma_start(
                        out=out[:, DV * (c - 1):DV * (c + 1)],
                        in_=outst[:, DV * (c - 1):DV * (c + 1)])

            # ---- schedule ----
            done = set()
            emitted = 0

            def pump():
                nonlocal emitted
                if emitted == 0 and {"t0", "t1", "k0", "q0"} <= done:
                    for c in range(cfg["chunk_grp"]):
                        run_chunk(c)
                    emitted = cfg["chunk_grp"]
                elif 0 < emitted < 4 and {"t0", "t1", "k0", "q0"} <= done:
                    for c in range(emitted, 4):
                        run_chunk(c)
                    emitted = 4
                if emitted == 4 and {"k1", "q1"} <= done:
                    for c in range(4, NCH):
                        run_chunk(c)
                    emitted = 8

            for step, blk in enumerate(cfg["phi_order"]):
                kind = {"t": "tok", "k": "k", "q": "q"}[blk[0]]
                j = int(blk[1])
                phi_block(kind, j, cfg["stt_eng"][step], step)
                if kind == "tok":
                    pre_s(j)
                elif kind == "k":
                    transposes(j)
                else:
                    a_mms(j)
                done.add(blk)
                pump()
            assert emitted == 8, "phi_order must cover t0,t1,k0,k1,q0,q1"
            if dbg is not None:
                nc.sync.dma_start(out=dbg[:, 0:H], in_=phit)
                nc.sync.dma_start(out=dbg[:, H:2 * H], in_=ktok)
                nc.sync.dma_start(
                    out=dbg[:, 2 * H:3 * H],
                    in_=kphi.rearrange("p f -> p f"))

    nc.compile()
    return nc


def _get_nc():
    if "nc" not in _cache:
        _cache["nc"] = _build(_cache.get("cfg"))
    return _cache["nc"]


def _pack_inputs(q, k, v, W_phi, b_phi):
    import ml_dtypes
    bf16 = ml_dtypes.bfloat16
    fp8 = ml_dtypes.float8_e4m3fn

    WT = np.ascontiguousarray(W_phi.T)                    # [d, e]
    maskm = np.triu(np.ones((C, C), np.float32))          # keep tau <= t
    ident = np.eye(C, dtype=np.float32)

    def aug(vh):  # [H, DV] -> [C, NCH*(DV+1)] partition-major with ones col
        a = np.concatenate([vh, np.ones((H, 1), np.float32)], axis=1)
        return a.reshape(NCH, C, VW).transpose(1, 0, 2).reshape(C, NCH * VW)

    zeros_vp = np.zeros((C, NCH * VW), np.float32)
    zeros_ktp = np.zeros((D, H), np.float32)

    in_maps = []
    for core in range(NCORES):
        b_idx, half = divmod(core, 2)
        sl = slice(half * H, (half + 1) * H)
        a8 = np.empty((D, B8_COLS), np.float32)
        a8[:, O8_WT:O8_WT + D] = WT
        a8[:, O8_QT:O8_QT + H] = q[b_idx, sl].T
        a8[:, O8_KT:O8_KT + H] = k[b_idx, sl].T
        a16 = np.empty((D, B16_COLS), np.float32)
        a16[:, O16_MASK:O16_MASK + C] = maskm
        a16[:, O16_ID:O16_ID + C] = ident
        a16[:, O16_BIAS] = b_phi
        if half == 1:
            a8[:, O8_KTP:O8_KTP + H] = k[b_idx, 0:H].T
            a16[:, O16_VP:O16_VP + NCH * VW] = aug(v[b_idx, 0:H])
        else:
            a8[:, O8_KTP:O8_KTP + H] = zeros_ktp
            a16[:, O16_VP:O16_VP + NCH * VW] = zeros_vp
        a16[:, O16_V:O16_V + NCH * VW] = aug(v[b_idx, sl])
        dt8 = fp8 if dict(CFG, **(_cache.get("cfg") or {}))["in_fp8"] else bf16
        bsm = np.concatenate([np.tile(b_phi, 4)[None, :],
                              np.ones((1, 512), np.float32)], axis=0)
        in_maps.append({"b8": a8.astype(dt8), "b16": a16.astype(bf16),
                        "bsm": bsm.astype(bf16)})
    return in_maps


def kernel(q, k, v, W_phi, b_phi):
    from concourse.bass_utils import run_bass_kernel_spmd

    q = np.asarray(q, np.float32)
    k = np.asarray(k, np.float32)
    v = np.asarray(v, np.float32)
    W_phi = np.asarray(W_phi, np.float32)
    b_phi = np.asarray(b_phi, np.float32)

    in_maps = _pack_inputs(q, k, v, W_phi, b_phi)
    nc = _get_nc()
    res = run_bass_kernel_spmd(nc, in_maps, list(range(NCORES)))

    out = np.empty((B, T, DV), np.float32)
    for core in range(NCORES):
        b_idx, half = divmod(core, 2)
        o = np.asarray(res.results[core]["out"], dtype=np.float32)
        o = o.reshape(C, NCH, DV).transpose(1, 0, 2).reshape(H, DV)
        out[b_idx, half * H:(half + 1) * H] = o
    return out


# revision 33
# speedup vs baseline: 1.1552x; 1.0777x over previous
"""Causal linear attention (ELU+1 feature map) on 8 trn2 NeuronCores. v2.

Sharding: core i handles batch b=i//2, sequence half h=i%2 (T=2048 -> 1024
tokens/core).  Second-half cores recompute the first half's running state
S0 = sum_tau phi(k_tau) [v_tau, 1]  (128x129, col 128 = z) from k/v of the
first half; first-half cores get zeroed aux inputs so their S0 == 0.

phi(y) = ELU(y)+1 = min(exp(y), max(y+1, 1))  (exact identity).
pre' = y + b + 1 computed in PSUM via matmul (bias row via ones-mm), then
  e = ACT Exp(pre' - 1);  phi = DVE stt: min(max(pre', 1), e).

q/k/W shipped fp8e4m3 (rel-err impact ~+7e-4), v/mask/ident bf16, out bf16.
"""

import numpy as np

B, T, D, DV = 4, 2048, 128, 128
H = T // 2          # tokens per core
C = 128             # chunk
NCH = H // C        # own chunks per core
NCORES = 8
VW = DV + 1

# fp8 pack: [WT | ktp | kT | qT]
O8_WT = 0
O8_KTP = O8_WT + D
O8_KT = O8_KTP + H
O8_QT = O8_KT + H
B8_COLS = O8_QT + H

# bf16 pack: [mask | ident | bias | vp | v]
O16_MASK = 0
O16_ID = O16_MASK + C
O16_BIAS = O16_ID + C
O16_VP = O16_BIAS + 1
O16_V = O16_VP + NCH * VW
B16_COLS = O16_V + NCH * VW

CFG = {
    "stt_eng": ("dve", "dve", "dve", "dve", "dve", "dve"),  # dve | poolsbuf
    "snap_eng": ("act",) * 8,
    "ktok_eng": ("dve", "dve"),   # per half copy
    "mask_eng": ("dve", "dve"),
    "scale_mode": "recip_dve",    # div_dve | recip_dve
    "dma_map": ("sync", "sync", "sync", "act", "sync", "gpsimd"),
    "out_dma": ("sync", "sync", "sync", "sync"),
    "phi_order": ("t0", "k0", "q0", "t1", "k1", "q1"),
    "chunk_grp": 2,
    "in_fp8": True,
    "out_bf16": True,
}

_cache = {}


def _build(cfg=None):
    import concourse.bacc as bacc
    import concourse.tile as tile
    from concourse import mybir
    from bass_rust import add_dep_helper

    cfg = dict(CFG, **(cfg or {}))
    F32 = mybir.dt.float32
    BF16 = mybir.dt.bfloat16
    FP8 = mybir.dt.float8e4
    AF = mybir.ActivationFunctionType
    ALU = mybir.AluOpType

    nc = bacc.Bacc(None, target_bir_lowering=False, debug=False,
                   num_devices=NCORES)

    DT8 = FP8 if cfg["in_fp8"] else BF16
    DTO = BF16 if cfg["out_bf16"] else F32
    b8 = nc.declare_dram_parameter("b8", [D, B8_COLS], DT8, isOutput=False)
    b16 = nc.declare_dram_parameter("b16", [D, B16_COLS], BF16,
                                    isOutput=False)
    bsm = nc.declare_dram_parameter("bsm", [2, 512], BF16, isOutput=False)
    out = nc.declare_dram_parameter("out", [C, NCH * DV], DTO, isOutput=True)
    dbg = nc.declare_dram_parameter("dbg", [C, 3 * H], BF16, isOutput=True) \
        if cfg.get("debug_phi") else None

    def eng(which):
        return {"dve": nc.vector, "act": nc.scalar, "pool": nc.gpsimd,
                "gpsimd": nc.gpsimd, "sync": nc.sync}[which]

    def copy_eng(which, dst, src):
        if which == "act":
            nc.scalar.activation(dst, src, AF.Copy)
        else:
            eng(which).tensor_copy(dst, src)

    with tile.TileContext(nc) as tc:
        with (
            tc.tile_pool(name="cst", bufs=1) as cst,
            tc.tile_pool(name="io", bufs=1) as io,
            tc.tile_pool(name="phi", bufs=1) as phip,
            tc.tile_pool(name="wrk", bufs=2) as wrk,
            tc.tile_pool(name="ps_pre", bufs=3, space="PSUM") as ps_pre,
            tc.tile_pool(name="ps_at", bufs=1, space="PSUM") as ps_at,
            tc.tile_pool(name="ps_s", bufs=1, space="PSUM") as ps_s,
            tc.tile_pool(name="ps_o", bufs=2, space="PSUM") as ps_o,
        ):
            # ---- warm the ACT table while DMAs run ----
            s_warm = cst.tile([D, 1], F32)
            nc.vector.memset(s_warm, 0.0)
            s_warm2 = cst.tile([D, 1], BF16)
            nc.scalar.activation(s_warm2, s_warm, AF.Exp)

            onesw = cst.tile([2, 512], BF16)
            nc.gpsimd.memset(onesw, 1.0)
            s_neg1 = cst.tile([D, 1], F32)
            nc.gpsimd.memset(s_neg1, -1.0)

            # ---- loads (need-ordered; SP carries the phi-critical fp8) ----
            s_b8 = io.tile([D, B8_COLS], DT8)
            s_b16 = io.tile([D, B16_COLS], BF16)
            sb2 = io.tile([2, 512], BF16)
            nc.gpsimd.dma_start(out=sb2[:, :], in_=bsm[:, :])
            HH = 512
            sp_pieces = [
                (0, O8_KTP + HH),                  # W + ktp half 0
                (O8_KT, O8_KT + HH),               # kT half 0
                (O8_QT, O8_QT + HH),               # qT half 0
                (O8_KTP + HH, O8_KT),              # ktp half 1
                (O8_KT + HH, O8_QT),               # kT half 1
                (O8_QT + HH, B8_COLS),             # qT half 1
            ]
            for a, b_ in sp_pieces:
                nc.sync.dma_start(out=s_b8[:, a:b_], in_=b8[:, a:b_])
            nc.scalar.dma_start(out=s_b16[:, 0:O16_VP],
                                in_=b16[:, 0:O16_VP])
            nc.scalar.dma_start(out=s_b16[:, O16_VP:O16_V],
                                in_=b16[:, O16_VP:O16_V])
            nc.gpsimd.dma_start(out=s_b16[:, O16_V:B16_COLS],
                                in_=b16[:, O16_V:B16_COLS])

            sWT = s_b8[:, O8_WT:O8_WT + D]
            s_mask = s_b16[:, O16_MASK:O16_MASK + C]
            s_ident = s_b16[:, O16_ID:O16_ID + C]
            s_bias = s_b16[:, O16_BIAS:O16_BIAS + 1]

            def vsl(c):
                return s_b16[:, O16_V + VW * c:O16_V + VW * (c + 1)]

            def vpsl(c):
                return s_b16[:, O16_VP + VW * c:O16_VP + VW * (c + 1)]




            # ---- state S: two parity accumulators ----
            S0 = ps_s.tile([D, VW], F32, tag="s0")
            S1 = ps_s.tile([D, VW], F32, tag="s1")
            Sp = [S0, S1]
            s_first = [None, None]
            started = [False, False]

            phit = phip.tile([C, H], BF16)      # phi(ktp) token-major
            kphi = phip.tile([D, H], BF16)      # phi(k) feature-major
            qphi = phip.tile([D, H], BF16)      # phi(q) feature-major
            ktok = phip.tile([C, H], BF16)      # phi(k) token-major
            Am = phip.tile([C, H], BF16)        # masked A, all chunks
            outst = phip.tile([C, NCH * DV], DTO)

            Ah = [None, None]

            def phi_block(kind, j, sidx, step=0):
                """kind: 'tok'|'k'|'q'; j: half index; sidx: stt engine."""
                pre = ps_pre.tile([D, 512], F32, tag="pre")
                if kind == "tok":
                    prev = nc.tensor.matmul(
                        pre, onesw[:, 0:C], sb2,
                        start=True, stop=False, skip_group_check=True)
                    for cc in range(4):
                        c = 4 * j + cc
                        mm = nc.tensor.matmul(
                            pre[:, C * cc:C * (cc + 1)],
                            s_b8[:, O8_KTP + C * c:O8_KTP + C * (c + 1)],
                            sWT, start=False, stop=(cc == 3),
                            skip_group_check=True)
                        add_dep_helper(mm.ins, prev.ins, sync=False,
                                       reason="psum group order")
                        prev = mm
                else:
                    off = O8_KT if kind == "k" else O8_QT
                    mm0 = nc.tensor.matmul(
                        pre, sWT, s_b8[:, off + 512 * j:off + 512 * (j + 1)],
                        start=True, stop=False, skip_group_check=True)
                    mm = nc.tensor.matmul(
                        pre, sb2[:, 0:C], onesw,
                        start=False, stop=True, skip_group_check=True)
                    add_dep_helper(mm.ins, mm0.ins, sync=False,
                                   reason="psum group order")
                e = phip.tile([D, 512], BF16, tag=f"e{step % 3}")
                nc.scalar.activation(e, pre, AF.Exp, bias=s_neg1, scale=1.0)
                dst = {"tok": phit, "k": kphi, "q": qphi}[kind]
                sl = slice(512 * j, 512 * (j + 1))
                if sidx == "poolsbuf":
                    # gpsimd cannot touch PSUM: relu via ACT, stt on SBUF
                    r = wrk.tile([D, 512], BF16, tag="r")
                    nc.scalar.activation(r, pre, AF.Relu, bias=s_neg1,
                                         scale=1.0)
                    nc.gpsimd.scalar_tensor_tensor(
                        out=dst[:, sl], in0=r, scalar=1.0, in1=e,
                        op0=ALU.add, op1=ALU.min)
                else:
                    eng(sidx).scalar_tensor_tensor(
                        out=dst[:, sl], in0=pre, scalar=1.0, in1=e,
                        op0=ALU.max, op1=ALU.min)

            def pre_s(j):
                for cc in range(4):
                    c = 4 * j + cc
                    p = c % 2
                    mm = nc.tensor.matmul(
                        Sp[p], phit[:, C * c:C * (c + 1)], vpsl(c),
                        start=(not started[p]), stop=False,
                        skip_group_check=True)
                    if started[p]:
                        add_dep_helper(mm.ins, s_first[p].ins, sync=False,
                                       reason="psum group order")
                    s_first[p] = mm
                    started[p] = True

            def transposes(h):
                trp_h = ps_at.tile([C, 512], BF16, tag="at")
                for cc in range(4):
                    c = 4 * h + cc
                    nc.tensor.transpose(trp_h[:, C * cc:C * (cc + 1)],
                                        kphi[:, C * c:C * (c + 1)], s_ident)
                sl = slice(512 * h, 512 * (h + 1))
                copy_eng(cfg["ktok_eng"][h], ktok[:, sl], trp_h)

            def a_mms(h):
                A = ps_at.tile([C, 512], F32, tag="at")
                Ah[h] = A
                for cc in range(4):
                    c = 4 * h + cc
                    nc.tensor.matmul(A[:, C * cc:C * (cc + 1)],
                                     kphi[:, C * c:C * (c + 1)],
                                     qphi[:, C * c:C * (c + 1)],
                                     start=True, stop=True,
                                     skip_group_check=True)
                sl = slice(512 * h, 512 * (h + 1))
                eng(cfg["mask_eng"][h]).tensor_tensor(
                    out=Am[:, sl].rearrange("p (c f) -> p c f", c=4),
                    in0=A.rearrange("p (c f) -> p c f", c=4),
                    in1=s_mask[:, None, :].broadcast_to([C, 4, C]),
                    op=ALU.mult)

            snaps = [None, None]
            Opair = [None]

            def run_chunk(c):
                pl = (0, 1) if c == 0 else ((c - 1) % 2,)
                for p in pl:
                    snap_t = wrk.tile([D, VW], BF16, tag=f"snap{p}")
                    snp = snap_t
                    snaps[p] = snp
                    copy_eng(cfg["snap_eng"][c], snp, Sp[p])

                if c % 2 == 0:
                    opair_t = ps_o.tile([C, 2 * VW], F32, tag="o")
                    Opair[0] = opair_t
                O = Opair[0]
                base = VW * (c % 2)
                osl = slice(base, base + VW)
                prev = nc.tensor.matmul(O[:, osl], Am[:, C * c:C * (c + 1)],
                                        vsl(c), start=True, stop=False,
                                        skip_group_check=True)
                for qi, sn in enumerate(snaps):
                    mm = nc.tensor.matmul(O[:, osl],
                                          qphi[:, C * c:C * (c + 1)], sn,
                                          start=False, stop=(qi == 1),
                                          skip_group_check=True)
                    add_dep_helper(mm.ins, prev.ins, sync=False,
                                   reason="psum group order")
                    prev = mm

                mm = nc.tensor.matmul(Sp[c % 2],
                                      ktok[:, C * c:C * (c + 1)], vsl(c),
                                      start=False,
                                      stop=(c >= NCH - 2),
                                      skip_group_check=True)
                add_dep_helper(mm.ins, s_first[c % 2].ins, sync=False,
                               reason="psum group order")
                s_first[c % 2] = mm

                if c % 2 == 1:
                    sm = cfg["scale_mode"]
                    for i, cc in enumerate((c - 1, c)):
                        bs = VW * i
                        if sm == "div_dve":
                            nc.vector.tensor_scalar(
                                out=outst[:, DV * cc:DV * (cc + 1)],
                                in0=O[:, bs:bs + DV],
                                scalar1=O[:, bs + DV:bs + DV + 1],
                                scalar2=None, op0=ALU.divide)
                        else:
                            rec = wrk.tile([C, 1], F32, tag="rec")
                            nc.vector.reciprocal(rec, O[:, bs + DV:bs + DV + 1])
                            nc.vector.tensor_scalar_mul(
                                outst[:, DV * cc:DV * (cc + 1)],
                                O[:, bs:bs + DV], rec)
                    oq = cfg["out_dma"][(c - 1) // 2]
                    eng(oq).dma_start(
                        out=out[:, DV * (c - 1):DV * (c + 1)],
                        in_=outst[:, DV * (c - 1):DV * (c + 1)])

            # ---- schedule ----
            done = set()
            emitted = 0

            def pump():
                nonlocal emitted
                if emitted == 0 and {"t0", "t1", "k0", "q0"} <= done:
                    for c in range(cfg["chunk_grp"]):
                        run_chunk(c)
                    emitted = cfg["chunk_grp"]
                elif 0 < emitted < 4 and {"t0", "t1", "k0", "q0"} <= done:
                    for c in range(emitted, 4):
                        run_chunk(c)
                    emitted = 4
                if emitted == 4 and {"k1", "q1"} <= done:
                    for c in range(4, NCH):
                        run_chunk(c)
                    emitted = 8

            for step, blk in enumerate(cfg["phi_order"]):
                kind = {"t": "tok", "k": "k", "q": "q"}[blk[0]]
                j = int(blk[1])
                phi_block(kind, j, cfg["stt_eng"][step], step)
                if kind == "tok":
                    pre_s(j)
                elif kind == "k":
                    transposes(j)
                else:
                    a_mms(j)
                done.add(blk)
                pump()
            assert emitted == 8, "phi_order must cover t0,t1,k0,k1,q0,q1"
            if dbg is not None:
                nc.sync.dma_start(out=dbg[:, 0:H], in_=phit)
                nc.sync.dma_start(out=dbg[:, H:2 * H], in_=ktok)
                nc.sync.dma_start(
                    out=dbg[:, 2 * H:3 * H],
                    in_=kphi.rearrange("p f -> p f"))

    nc.compile()
    return nc


def _get_nc():
    if "nc" not in _cache:
        _cache["nc"] = _build(_cache.get("cfg"))
    return _cache["nc"]


def _pack_inputs(q, k, v, W_phi, b_phi):
    import ml_dtypes
    bf16 = ml_dtypes.bfloat16
    fp8 = ml_dtypes.float8_e4m3fn

    WT = np.ascontiguousarray(W_phi.T)                    # [d, e]
    maskm = np.triu(np.ones((C, C), np.float32))          # keep tau <= t
    ident = np.eye(C, dtype=np.float32)

    def aug(vh):  # [H, DV] -> [C, NCH*(DV+1)] partition-major with ones col
        a = np.concatenate([vh, np.ones((H, 1), np.float32)], axis=1)
        return a.reshape(NCH, C, VW).transpose(1, 0, 2).reshape(C, NCH * VW)

    zeros_vp = np.zeros((C, NCH * VW), np.float32)
    zeros_ktp = np.zeros((D, H), np.float32)

    in_maps = []
    for core in range(NCORES):
        b_idx, half = divmod(core, 2)
        sl = slice(half * H, (half + 1) * H)
        a8 = np.empty((D, B8_COLS), np.float32)
        a8[:, O8_WT:O8_WT + D] = WT
        a8[:, O8_QT:O8_QT + H] = q[b_idx, sl].T
        a8[:, O8_KT:O8_KT + H] = k[b_idx, sl].T
        a16 = np.empty((D, B16_COLS), np.float32)
        a16[:, O16_MASK:O16_MASK + C] = maskm
        a16[:, O16_ID:O16_ID + C] = ident
        a16[:, O16_BIAS] = b_phi
        if half == 1:
            a8[:, O8_KTP:O8_KTP + H] = k[b_idx, 0:H].T
            a16[:, O16_VP:O16_VP + NCH * VW] = aug(v[b_idx, 0:H])
        else:
            a8[:, O8_KTP:O8_KTP + H] = zeros_ktp
            a16[:, O16_VP:O16_VP + NCH * VW] = zeros_vp
        a16[:, O16_V:O16_V + NCH * VW] = aug(v[b_idx, sl])
        dt8 = fp8 if dict(CFG, **(_cache.get("cfg") or {}))["in_fp8"] else bf16
        bsm = np.concatenate([np.tile(b_phi, 4)[None, :],
                              np.ones((1, 512), np.float32)], axis=0)
        in_maps.append({"b8": a8.astype(dt8), "b16": a16.astype(bf16),
                        "bsm": bsm.astype(bf16)})
    return in_maps


def kernel(q, k, v, W_phi, b_phi):
    from concourse.bass_utils import run_bass_kernel_spmd

    q = np.asarray(q, np.float32)
    k = np.asarray(k, np.float32)
    v = np.asarray(v, np.float32)
    W_phi = np.asarray(W_phi, np.float32)
    b_phi = np.asarray(b_phi, np.float32)

    in_maps = _pack_inputs(q, k, v, W_phi, b_phi)
    nc = _get_nc()
    res = run_bass_kernel_spmd(nc, in_maps, list(range(NCORES)))

    out = np.empty((B, T, DV), np.float32)
    for core in range(NCORES):
        b_idx, half = divmod(core, 2)
        o = np.asarray(res.results[core]["out"], dtype=np.float32)
        o = o.reshape(C, NCH, DV).transpose(1, 0, 2).reshape(H, DV)
        out[b_idx, half * H:(half + 1) * H] = o
    return out


# revision 39
# speedup vs baseline: 1.1960x; 1.0353x over previous
"""Causal linear attention (ELU+1 feature map) on 8 trn2 NeuronCores. v2.

Sharding: core i handles batch b=i//2, sequence half h=i%2 (T=2048 -> 1024
tokens/core).  Second-half cores recompute the first half's running state
S0 = sum_tau phi(k_tau) [v_tau, 1]  (128x129, col 128 = z) from k/v of the
first half; first-half cores get zeroed aux inputs so their S0 == 0.

phi(y) = ELU(y)+1 = min(exp(y), max(y+1, 1))  (exact identity).
pre' = y + b + 1 computed in PSUM via matmul (bias row via ones-mm), then
  e = ACT Exp(pre' - 1);  phi = DVE stt: min(max(pre', 1), e).

q/k/W shipped fp8e4m3 (rel-err impact ~+7e-4), v/mask/ident bf16, out bf16.
"""

import numpy as np

B, T, D, DV = 4, 2048, 128, 128
H = T // 2          # tokens per core
C = 128             # chunk
NCH = H // C        # own chunks per core
NCORES = 8
VW = DV + 1

# fp8 pack: [WT | ktp | kT | qT]
O8_WT = 0
O8_KTP = O8_WT + D
O8_KT = O8_KTP + H
O8_QT = O8_KT + H
B8_COLS = O8_QT + H

# bf16 pack: [mask3 (triu|ones|triu) | ident | bias | vp | v]
O16_MASK = 0
O16_ID = O16_MASK + 3 * C
O16_BIAS = O16_ID + C
O16_VP = O16_BIAS + 1
O16_V = O16_VP + NCH * VW
B16_COLS = O16_V + NCH * VW

CFG = {
    "stt_eng": ("dve", "dve", "dve", "dve", "dve", "dve"),  # dve | poolsbuf
    "snap_eng": ("act",) * 4,
    "ktok_eng": ("dve", "dve"),   # per half copy
    "mask_eng": ("dve", "dve"),
    "scale_mode": "recip_dve",    # div_dve | recip_dve
    "dma_map": ("sync", "sync", "sync", "act", "sync", "gpsimd"),
    "out_dma": ("sync", "sync", "sync", "sync"),
    "phi_order": ("t0", "k0", "q0", "t1", "k1", "q1"),
    "chunk_grp": 2,
    "pump_mode": "interleave",    # interleave | after_q1
    "bsm_q": "act",
    "out_mode": "pair",           # pair | p53 | p35
    "in_fp8": True,
    "out_bf16": True,
}

_cache = {}


def _build(cfg=None):
    import concourse.bacc as bacc
    import concourse.tile as tile
    from concourse import mybir
    from bass_rust import add_dep_helper

    cfg = dict(CFG, **(cfg or {}))
    F32 = mybir.dt.float32
    BF16 = mybir.dt.bfloat16
    FP8 = mybir.dt.float8e4
    AF = mybir.ActivationFunctionType
    ALU = mybir.AluOpType

    nc = bacc.Bacc(None, target_bir_lowering=False, debug=False,
                   num_devices=NCORES)

    DT8 = FP8 if cfg["in_fp8"] else BF16
    DTO = BF16 if cfg["out_bf16"] else F32
    b8 = nc.declare_dram_parameter("b8", [D, B8_COLS], DT8, isOutput=False)
    b16 = nc.declare_dram_parameter("b16", [D, B16_COLS], BF16,
                                    isOutput=False)
    bsm = nc.declare_dram_parameter("bsm", [2, 512], BF16, isOutput=False)
    out = nc.declare_dram_parameter("out", [C, NCH * DV], DTO, isOutput=True)
    dbg = nc.declare_dram_parameter("dbg", [C, 3 * H], BF16, isOutput=True) \
        if cfg.get("debug_phi") else None

    def eng(which):
        return {"dve": nc.vector, "act": nc.scalar, "pool": nc.gpsimd,
                "gpsimd": nc.gpsimd, "sync": nc.sync}[which]

    def copy_eng(which, dst, src):
        if which == "act":
            nc.scalar.activation(dst, src, AF.Copy)
        else:
            eng(which).tensor_copy(dst, src)

    with tile.TileContext(nc) as tc:
        with (
            tc.tile_pool(name="cst", bufs=1) as cst,
            tc.tile_pool(name="io", bufs=1) as io,
            tc.tile_pool(name="phi", bufs=1) as phip,
            tc.tile_pool(name="wrk", bufs=2) as wrk,
            tc.tile_pool(name="ps_pre", bufs=3, space="PSUM") as ps_pre,
            tc.tile_pool(name="ps_at", bufs=1, space="PSUM") as ps_at,
            tc.tile_pool(name="ps_s", bufs=1, space="PSUM") as ps_s,
            tc.tile_pool(name="ps_o", bufs=2, space="PSUM") as ps_o,
        ):
            # ---- warm the ACT table while DMAs run ----
            s_warm = cst.tile([D, 1], F32)
            nc.vector.memset(s_warm, 0.0)
            s_warm2 = cst.tile([D, 1], BF16)
            nc.scalar.activation(s_warm2, s_warm, AF.Exp)

            onesw = cst.tile([2, 512], BF16)
            nc.gpsimd.memset(onesw, 1.0)
            s_neg1 = cst.tile([D, 1], F32)
            nc.gpsimd.memset(s_neg1, -1.0)

            # ---- loads (need-ordered; SP carries the phi-critical fp8) ----
            s_b8 = io.tile([D, B8_COLS], DT8)
            s_b16 = io.tile([D, B16_COLS], BF16)
            sb2 = io.tile([2, 512], BF16)
            eng(cfg["bsm_q"]).dma_start(out=sb2[:, :], in_=bsm[:, :])
            HH = 512
            sp_pieces = [
                (0, O8_KTP + HH),                  # W + ktp half 0
                (O8_KT, O8_KT + HH),               # kT half 0
                (O8_QT, O8_QT + HH),               # qT half 0
                (O8_KTP + HH, O8_KT),              # ktp half 1
                (O8_KT + HH, O8_QT),               # kT half 1
                (O8_QT + HH, B8_COLS),             # qT half 1
            ]
            for a, b_ in sp_pieces:
                nc.sync.dma_start(out=s_b8[:, a:b_], in_=b8[:, a:b_])
            nc.scalar.dma_start(out=s_b16[:, 0:O16_VP],
                                in_=b16[:, 0:O16_VP])
            nc.scalar.dma_start(out=s_b16[:, O16_VP:O16_V],
                                in_=b16[:, O16_VP:O16_V])
            nc.gpsimd.dma_start(out=s_b16[:, O16_V:B16_COLS],
                                in_=b16[:, O16_V:B16_COLS])

            sWT = s_b8[:, O8_WT:O8_WT + D]
            s_mask3 = s_b16[:, O16_MASK:O16_MASK + 3 * C]
            s_ident = s_b16[:, O16_ID:O16_ID + C]
            s_bias = s_b16[:, O16_BIAS:O16_BIAS + 1]

            def vsl(c):
                return s_b16[:, O16_V + VW * c:O16_V + VW * (c + 1)]

            def vpsl(c):
                return s_b16[:, O16_VP + VW * c:O16_VP + VW * (c + 1)]




            # ---- state S: single accumulator (pair-snapshot scheme) ----
            S0 = ps_s.tile([D, VW], F32, tag="s0")
            s_state = [None, False]   # [last mm, started]

            phit = phip.tile([C, H], BF16)      # phi(ktp) token-major
            kphi = phip.tile([D, H], BF16)      # phi(k) feature-major
            qphi = phip.tile([D, H], BF16)      # phi(q) feature-major
            ktok = phip.tile([C, H], BF16)      # phi(k) token-major
            Am = phip.tile([C, 4 * 3 * C], BF16)  # [A|B|A'] per pair
            outst = phip.tile([C, NCH * DV], DTO)

            Ah = [None, None]

            def phi_block(kind, j, sidx, step=0):
                """kind: 'tok'|'k'|'q'; j: half index; sidx: stt engine."""
                pre = ps_pre.tile([D, 512], F32, tag="pre")
                if kind == "tok":
                    prev = nc.tensor.matmul(
                        pre, onesw[:, 0:C], sb2,
                        start=True, stop=False, skip_group_check=True)
                    for cc in range(4):
                        c = 4 * j + cc
                        mm = nc.tensor.matmul(
                            pre[:, C * cc:C * (cc + 1)],
                            s_b8[:, O8_KTP + C * c:O8_KTP + C * (c + 1)],
                            sWT, start=False, stop=(cc == 3),
                            skip_group_check=True)
                        add_dep_helper(mm.ins, prev.ins, sync=False,
                                       reason="psum group order")
                        prev = mm
                else:
                    off = O8_KT if kind == "k" else O8_QT
                    mm0 = nc.tensor.matmul(
                        pre, sWT, s_b8[:, off + 512 * j:off + 512 * (j + 1)],
                        start=True, stop=False, skip_group_check=True)
                    mm = nc.tensor.matmul(
                        pre, sb2[:, 0:C], onesw,
                        start=False, stop=True, skip_group_check=True)
                    add_dep_helper(mm.ins, mm0.ins, sync=False,
                                   reason="psum group order")
                e = phip.tile([D, 512], BF16, tag=f"e{step % 3}")
                nc.scalar.activation(e, pre, AF.Exp, bias=s_neg1, scale=1.0)
                dst = {"tok": phit, "k": kphi, "q": qphi}[kind]
                sl = slice(512 * j, 512 * (j + 1))
                if sidx == "poolsbuf":
                    # gpsimd cannot touch PSUM: relu via ACT, stt on SBUF
                    r = wrk.tile([D, 512], BF16, tag="r")
                    nc.scalar.activation(r, pre, AF.Relu, bias=s_neg1,
                                         scale=1.0)
                    nc.gpsimd.scalar_tensor_tensor(
                        out=dst[:, sl], in0=r, scalar=1.0, in1=e,
                        op0=ALU.add, op1=ALU.min)
                else:
                    eng(sidx).scalar_tensor_tensor(
                        out=dst[:, sl], in0=pre, scalar=1.0, in1=e,
                        op0=ALU.max, op1=ALU.min)

            def pre_s(j):
                for cc in range(4):
                    c = 4 * j + cc
                    mm = nc.tensor.matmul(
                        S0, phit[:, C * c:C * (c + 1)], vpsl(c),
                        start=(not s_state[1]), stop=False,
                        skip_group_check=True)
                    if s_state[1]:
                        add_dep_helper(mm.ins, s_state[0].ins, sync=False,
                                       reason="psum group order")
                    s_state[0] = mm
                    s_state[1] = True

            def transposes(h):
                trp_h = ps_at.tile([C, 512], BF16, tag="at")
                for cc in range(4):
                    c = 4 * h + cc
                    nc.tensor.transpose(trp_h[:, C * cc:C * (cc + 1)],
                                        kphi[:, C * c:C * (c + 1)], s_ident)
                sl = slice(512 * h, 512 * (h + 1))
                copy_eng(cfg["ktok_eng"][h], ktok[:, sl], trp_h)

            def a_mms(h):
                # per pair p (chunks c=2p, c+1): blocks [A(c,c)|B(c,c+1)|A(c+1,c+1)]
                A = ps_at.tile([C, 3 * 2 * C], F32, tag="at")
                Ah[h] = A
                for pp in range(2):
                    p = 2 * h + pp
                    c = 2 * p
                    base = 3 * C * pp
                    nc.tensor.matmul(A[:, base:base + C],
                                     kphi[:, C * c:C * (c + 1)],
                                     qphi[:, C * c:C * (c + 1)],
                                     start=True, stop=True,
                                     skip_group_check=True)
                    nc.tensor.matmul(A[:, base + C:base + 2 * C],
                                     kphi[:, C * c:C * (c + 1)],
                                     qphi[:, C * (c + 1):C * (c + 2)],
                                     start=True, stop=True,
                                     skip_group_check=True)
                    nc.tensor.matmul(A[:, base + 2 * C:base + 3 * C],
                                     kphi[:, C * (c + 1):C * (c + 2)],
                                     qphi[:, C * (c + 1):C * (c + 2)],
                                     start=True, stop=True,
                                     skip_group_check=True)
                sl = slice(2 * 3 * C * h, 2 * 3 * C * (h + 1))
                eng(cfg["mask_eng"][h]).tensor_tensor(
                    out=Am[:, sl].rearrange("p (c f) -> p c f", c=2),
                    in0=A.rearrange("p (c f) -> p c f", c=2),
                    in1=s_mask3[:, None, :].broadcast_to([C, 2, 3 * C]),
                    op=ALU.mult)

            def run_pair(p):
                c = 2 * p
                snap_t = wrk.tile([D, VW], BF16, tag="snap")
                snp = snap_t
                copy_eng(cfg["snap_eng"][p], snp, S0)

                opair_t = ps_o.tile([C, 2 * VW], F32, tag="o")
                O = opair_t
                ab = 3 * C * p
                # chunk c
                m1 = nc.tensor.matmul(O[:, 0:VW], Am[:, ab:ab + C],
                                      vsl(c), start=True, stop=False,
                                      skip_group_check=True)
                m2 = nc.tensor.matmul(O[:, 0:VW], qphi[:, C * c:C * (c + 1)],
                                      snp, start=False, stop=True,
                                      skip_group_check=True)
                add_dep_helper(m2.ins, m1.ins, sync=False,
                               reason="psum group order")
                # chunk c+1: intra + cross-B + state
                m3 = nc.tensor.matmul(O[:, VW:2 * VW],
                                      Am[:, ab + 2 * C:ab + 3 * C],
                                      vsl(c + 1), start=True, stop=False,
                                      skip_group_check=True)
                m4 = nc.tensor.matmul(O[:, VW:2 * VW],
                                      Am[:, ab + C:ab + 2 * C],
                                      vsl(c), start=False, stop=False,
                                      skip_group_check=True)
                add_dep_helper(m4.ins, m3.ins, sync=False,
                               reason="psum group order")
                m5 = nc.tensor.matmul(O[:, VW:2 * VW],
                                      qphi[:, C * (c + 1):C * (c + 2)],
                                      snp, start=False, stop=True,
                                      skip_group_check=True)
                add_dep_helper(m5.ins, m4.ins, sync=False,
                               reason="psum group order")

                for cc in (c, c + 1):
                    mm = nc.tensor.matmul(S0, ktok[:, C * cc:C * (cc + 1)],
                                          vsl(cc), start=False,
                                          stop=(cc == NCH - 1),
                                          skip_group_check=True)
                    add_dep_helper(mm.ins, s_state[0].ins, sync=False,
                                   reason="psum group order")
                    s_state[0] = mm

                for i, cc in enumerate((c, c + 1)):
                    bs = VW * i
                    rec = wrk.tile([C, 1], F32, tag="rec")
                    nc.vector.reciprocal(rec, O[:, bs + DV:bs + DV + 1])
                    nc.vector.tensor_scalar_mul(
                        outst[:, DV * cc:DV * (cc + 1)],
                        O[:, bs:bs + DV], rec)
                om = cfg["out_mode"]
                if om == "pair":
                    oq = cfg["out_dma"][p]
                    eng(oq).dma_start(
                        out=out[:, DV * c:DV * (c + 2)],
                        in_=outst[:, DV * c:DV * (c + 2)])
                elif om == "p53" and p in (2, 3):
                    a_, b2 = (0, 6 * DV) if p == 2 else (6 * DV, 8 * DV)
                    eng(cfg["out_dma"][0]).dma_start(
                        out=out[:, a_:b2], in_=outst[:, a_:b2])
                elif om == "p35" and p in (1, 3):
                    a_, b2 = (0, 4 * DV) if p == 1 else (4 * DV, 8 * DV)
                    eng(cfg["out_dma"][0]).dma_start(
                        out=out[:, a_:b2], in_=outst[:, a_:b2])

            # ---- schedule ----
            done = set()
            emitted = 0

            def pump():
                nonlocal emitted
                if cfg["pump_mode"] == "after_q1":
                    if len(done) == len(cfg["phi_order"]):
                        for p in range(4):
                            run_pair(p)
                        emitted = 8
                    return
                if emitted == 0 and {"t0", "t1", "k0", "q0"} <= done:
                    for p in range(2):
                        run_pair(p)
                    emitted = 4
                if emitted == 4 and {"k1", "q1"} <= done:
                    for p in range(2, 4):
                        run_pair(p)
                    emitted = 8

            for step, blk in enumerate(cfg["phi_order"]):
                kind = {"t": "tok", "k": "k", "q": "q"}[blk[0]]
                j = int(blk[1])
                phi_block(kind, j, cfg["stt_eng"][step], step)
                if kind == "tok":
                    pre_s(j)
                elif kind == "k":
                    transposes(j)
                else:
                    a_mms(j)
                done.add(blk)
                pump()
            assert emitted == 8, "phi_order must cover t0,t1,k0,k1,q0,q1"
            if dbg is not None:
                nc.sync.dma_start(out=dbg[:, 0:H], in_=phit)
                nc.sync.dma_start(out=dbg[:, H:2 * H], in_=ktok)
                nc.sync.dma_start(
                    out=dbg[:, 2 * H:3 * H],
                    in_=kphi.rearrange("p f -> p f"))

    nc.compile()
    return nc


def _get_nc():
    if "nc" not in _cache:
        _cache["nc"] = _build(_cache.get("cfg"))
    return _cache["nc"]


def _pack_inputs(q, k, v, W_phi, b_phi):
    import ml_dtypes
    bf16 = ml_dtypes.bfloat16
    fp8 = ml_dtypes.float8_e4m3fn

    WT = np.ascontiguousarray(W_phi.T)                    # [d, e]
    maskm = np.triu(np.ones((C, C), np.float32))          # keep tau <= t
    ident = np.eye(C, dtype=np.float32)

    def aug(vh):  # [H, DV] -> [C, NCH*(DV+1)] partition-major with ones col
        a = np.concatenate([vh, np.ones((H, 1), np.float32)], axis=1)
        return a.reshape(NCH, C, VW).transpose(1, 0, 2).reshape(C, NCH * VW)

    zeros_vp = np.zeros((C, NCH * VW), np.float32)
    zeros_ktp = np.zeros((D, H), np.float32)

    in_maps = []
    for core in range(NCORES):
        b_idx, half = divmod(core, 2)
        sl = slice(half * H, (half + 1) * H)
        a8 = np.empty((D, B8_COLS), np.float32)
        a8[:, O8_WT:O8_WT + D] = WT
        a8[:, O8_QT:O8_QT + H] = q[b_idx, sl].T
        a8[:, O8_KT:O8_KT + H] = k[b_idx, sl].T
        a16 = np.empty((D, B16_COLS), np.float32)
        a16[:, O16_MASK:O16_MASK + C] = maskm
        a16[:, O16_MASK + C:O16_MASK + 2 * C] = 1.0
        a16[:, O16_MASK + 2 * C:O16_MASK + 3 * C] = maskm
        a16[:, O16_ID:O16_ID + C] = ident
        a16[:, O16_BIAS] = b_phi
        if half == 1:
            a8[:, O8_KTP:O8_KTP + H] = k[b_idx, 0:H].T
            a16[:, O16_VP:O16_VP + NCH * VW] = aug(v[b_idx, 0:H])
        else:
            a8[:, O8_KTP:O8_KTP + H] = zeros_ktp
            a16[:, O16_VP:O16_VP + NCH * VW] = zeros_vp
        a16[:, O16_V:O16_V + NCH * VW] = aug(v[b_idx, sl])
        dt8 = fp8 if dict(CFG, **(_cache.get("cfg") or {}))["in_fp8"] else bf16
        bsm = np.concatenate([np.tile(b_phi, 4)[None, :],
                              np.ones((1, 512), np.float32)], axis=0)
        in_maps.append({"b8": a8.astype(dt8), "b16": a16.astype(bf16),
                        "bsm": bsm.astype(bf16)})
    return in_maps


def kernel(q, k, v, W_phi, b_phi):
    from concourse.bass_utils import run_bass_kernel_spmd

    q = np.asarray(q, np.float32)
    k = np.asarray(k, np.float32)
    v = np.asarray(v, np.float32)
    W_phi = np.asarray(W_phi, np.float32)
    b_phi = np.asarray(b_phi, np.float32)

    in_maps = _pack_inputs(q, k, v, W_phi, b_phi)
    nc = _get_nc()
    res = run_bass_kernel_spmd(nc, in_maps, list(range(NCORES)))

    out = np.empty((B, T, DV), np.float32)
    for core in range(NCORES):
        b_idx, half = divmod(core, 2)
        o = np.asarray(res.results[core]["out"], dtype=np.float32)
        o = o.reshape(C, NCH, DV).transpose(1, 0, 2).reshape(H, DV)
        out[b_idx, half * H:(half + 1) * H] = o
    return out
